# revision 1
# baseline (speedup 1.0000x reference)
"""AiLUT forward on 8 trn2 NeuronCores.

Sharding: pure data parallel — batch 4 images x 2-way spatial split of the
per-pixel LUT transform = 8 shards. Each device computes the backbone +
LUT/vertex generation for its image (replicated across the image's two
devices) and applies the per-pixel transform to its half of the image.

The per-pixel stage is restructured to be XLA/neuron friendly:
  - searchsorted  -> 31 broadcast compares + sum (exact, no sort primitive)
  - trilinear LUT -> per-cell monomial-coefficient table (built once per
    image with static strided slices) + ONE flat gather of 24 coeffs per
    pixel + a fused polynomial Horner evaluation.
"""

import numpy as np
import jax
import jax.numpy as jnp
from jax import lax
from jax.sharding import Mesh, PartitionSpec
from jax.experimental.shard_map import shard_map
from functools import partial

D = 33
NEG_SLOPE = 0.2
EPS_IN = 1e-5

# ----------------------------------------------------------------- backbone


def _block(x, w, b, gamma, beta):
    y = lax.conv_general_dilated(x, w, (2, 2), [(1, 1), (1, 1)],
                                 dimension_numbers=('NCHW', 'OIHW', 'NCHW'))
    y = y + b[None, :, None, None]
    y = jnp.where(y >= 0, y, NEG_SLOPE * y)
    if gamma is not None:
        mu = jnp.mean(y, (2, 3), keepdims=True)
        var = jnp.var(y, (2, 3), keepdims=True)
        y = (y - mu) * lax.rsqrt(var + EPS_IN) * gamma[None, :, None, None] \
            + beta[None, :, None, None]
    return y


def _backbone_one(lq, p):
    # lq [3,1024,1024] -> codes [512]
    # F.interpolate(1024->256, bilinear, half-pixel): exact = mean of the
    # 2x2 block at offset (1,1) of each 4x4 block.
    x = lq.reshape(3, 256, 4, 256, 4)
    x = 0.25 * (x[:, :, 1, :, 1] + x[:, :, 1, :, 2]
                + x[:, :, 2, :, 1] + x[:, :, 2, :, 2])
    x = x[None]                                     # [1,3,256,256]
    x = _block(x, p['w1'], p['b1'], p['g1'], p['be1'])
    x = _block(x, p['w2'], p['b2'], p['g2'], p['be2'])
    x = _block(x, p['w3'], p['b3'], p['g3'], p['be3'])
    x = _block(x, p['w4'], p['b4'], p['g4'], p['be4'])
    x = _block(x, p['w5'], p['b5'], None, None)     # [1,128,8,8]
    x = x.reshape(1, 128, 2, 4, 2, 4).mean((3, 5))  # [1,128,2,2]
    return x.reshape(-1)                            # [512]


# ------------------------------------------------------- per-image transform


def _cell_coeff_table(lut, verts):
    """lut [3,33,33,33], verts [3,33] ->
       alpha/beta per axis per cell [3,32] and M [3,8,32768] monomial coeffs
       (frac basis) per output channel."""
    dv = verts[:, 1:] - verts[:, :-1]               # [3,32]
    beta = 1.0 / (dv + 1e-8)
    alpha = -verts[:, :-1] * beta                   # fr = alpha + beta*p

    c = lut                                          # [3,33,33,33]
    V = {}
    for a in (0, 1):
        for b in (0, 1):
            for e in (0, 1):
                V[(a, b, e)] = c[:, a:a + 32, b:b + 32, e:e + 32]
    T000 = V[(0, 0, 0)]
    T100 = V[(1, 0, 0)] - T000
    T010 = V[(0, 1, 0)] - T000
    T001 = V[(0, 0, 1)] - T000
    T110 = V[(1, 1, 0)] - V[(1, 0, 0)] - V[(0, 1, 0)] + T000
    T101 = V[(1, 0, 1)] - V[(1, 0, 0)] - V[(0, 0, 1)] + T000
    T011 = V[(0, 1, 1)] - V[(0, 1, 0)] - V[(0, 0, 1)] + T000
    T111 = (V[(1, 1, 1)] - V[(1, 1, 0)] - V[(1, 0, 1)] - V[(0, 1, 1)]
            + V[(1, 0, 0)] + V[(0, 1, 0)] + V[(0, 0, 1)] - T000)
    # order: index by (a<<2)|(b<<1)|e
    M = jnp.stack([T000, T001, T010, T011, T100, T101, T110, T111], axis=1)
    return alpha, beta, M.reshape(3, 8, 32 * 32 * 32)


def _searchsorted_cells(p, verts):
    """p [3,N] pixel values; verts [3,33] sorted. Return cell index i0 [3,N]
    (= searchsorted(v, p, right) - 1 clipped to [0,31]) exactly."""
    # i0 = sum_{k=1..31} (v_k <= p); v_0 = 0 <= p always, v_32 = 1 > p.
    acc = jnp.zeros(p.shape, jnp.int32)
    for k in range(1, 32):
        acc = acc + (verts[:, k:k + 1] <= p).astype(jnp.int32)
    return acc


def _transform_half(lqh, lut, verts):
    """lqh [3,N] half-image pixels; returns [3,N] transformed (pre-clip)."""
    N = lqh.shape[1]
    i0 = _searchsorted_cells(lqh, verts)            # [3,N] in [0,31]
    alpha, beta, M = _cell_coeff_table(lut, verts)
    fa = jnp.take_along_axis(alpha, i0, axis=1)     # [3,N]
    fb = jnp.take_along_axis(beta, i0, axis=1)
    fr = fa + fb * lqh                              # frac coords [3,N]
    key = (i0[0] * 1024 + i0[1] * 32 + i0[2])       # [N]
    # gather 8 coeffs x 3 channels
    C = M[:, :, key]                                # [3,8,N]
    f_r, f_g, f_b = fr[0], fr[1], fr[2]
    q = f_g * f_b
    lo = C[:, 0] + C[:, 1] * f_b + C[:, 2] * f_g + C[:, 3] * q
    hi = C[:, 4] + C[:, 5] * f_b + C[:, 6] * f_g + C[:, 7] * q
    out = lo + f_r * hi                             # [3,N]
    return jnp.clip(out, 0.0, 1.0)


# ------------------------------------------------------------------- driver


_PARAM_NAMES = ['w1', 'b1', 'g1', 'be1', 'w2', 'b2', 'g2', 'be2',
                'w3', 'b3', 'g3', 'be3', 'w4', 'b4', 'g4', 'be4',
                'w5', 'b5', 'lw', 'lb', 'bw', 'aw', 'ab']

_compiled = {}


def _device_fn(lq_full, half, params):
    """Per-device: lq_full [3,1024,1024] (this device's image), half [] int32
    (0: rows 0..511, 1: rows 512..1023), params dict (replicated)."""
    codes = _backbone_one(lq_full, params)                    # [512]
    weights = codes @ params['lw'].T + params['lb']           # [3]
    lut = (weights @ params['bw'].T).reshape(3, D, D, D)
    intervals = (codes @ params['aw'].T + params['ab']).reshape(3, D - 1)
    intervals = jax.nn.softmax(intervals, axis=-1)
    verts = jnp.pad(jnp.cumsum(intervals, axis=-1), ((0, 0), (1, 0)))  # [3,33]

    lqh = lax.dynamic_slice(lq_full, (0, half * 512, 0), (3, 512, 1024))
    out_h = _transform_half(lqh.reshape(3, -1), lut, verts)
    return out_h.reshape(3, 512, 1024), weights, verts


def _build(devs):
    mesh = Mesh(np.asarray(devs), ('core',))

    def spmd(lq8, half8, params):
        # shapes inside shard_map: lq8 [1,3,1024,1024], half8 [1]
        o, w, v = _device_fn(lq8[0], half8[0], params)
        return o[None], w[None], v[None]

    fn = shard_map(spmd, mesh=mesh,
                   in_specs=(PartitionSpec('core'), PartitionSpec('core'),
                             PartitionSpec()),
                   out_specs=(PartitionSpec('core'), PartitionSpec('core'),
                              PartitionSpec('core')))
    return jax.jit(fn)


def kernel(**inputs):
    lq = np.asarray(inputs['lq'], np.float32)       # [4,3,1024,1024]
    B = lq.shape[0]
    params = {k: jnp.asarray(np.asarray(inputs[k], np.float32))
              for k in _PARAM_NAMES}

    devs = jax.devices()[:8]
    key = 'k8'
    if key not in _compiled:
        _compiled[key] = _build(devs)
    fn = _compiled[key]

    # device d handles image d//2, half d%2
    lq8 = np.stack([lq[d // 2] for d in range(8)])            # [8,3,1024,1024]
    half8 = np.array([d % 2 for d in range(8)], np.int32)

    o8, w8, v8 = fn(jnp.asarray(lq8), jnp.asarray(half8), params)
    o8 = np.asarray(o8)                                        # [8,3,512,1024]
    outs = np.empty((B, 3, 1024, 1024), np.float32)
    for b in range(B):
        outs[b, :, :512] = o8[2 * b]
        outs[b, :, 512:] = o8[2 * b + 1]
    weights = np.asarray(w8)[::2][:B]                          # [4,3]
    vertices = np.asarray(v8)[::2][:B]                         # [4,3,33]
    return outs, weights, vertices
